# revision 1
# baseline (speedup 1.0000x reference)
"""Trainium2 Bass kernel for nn_Block_27848567948000 (dense transformer block).

Sharding (8 NeuronCores): 4 data-parallel groups over batch (B=4), 2-way
tensor-parallel within each pair: attention sharded over heads (5 each),
out_proj/MLP sharded over tokens (1024 each) after an 8-rank AllGather
exchange of attention outputs.

kernel(**inputs) takes FULL inputs and returns the FULL (4, 2048, 1280) output.
"""
import sys
import os

sys.path.insert(0, '/opt/trn_rl_repo')

import numpy as np
import ml_dtypes

import concourse.bass as bass
import concourse.tile as tile
from concourse import mybir, bacc
from concourse import bass_utils
from concourse.masks import make_identity

B, T, C, H, D, F = 4, 2048, 1280, 10, 128, 5120
EPS = 1e-5
N_CORES = 8
HPC = H // 2            # heads per core
CPC = HPC * D           # channels per core (640)
f32 = mybir.dt.float32
f32r = mybir.dt.float32r
bf16 = mybir.dt.bfloat16
fp8 = mybir.dt.float8e4
i32 = mybir.dt.int32
AF = mybir.ActivationFunctionType
OP = mybir.AluOpType
AX = mybir.AxisListType


def _bcast(ap, n=128):
    """Broadcast a (1, k) DRAM/SBUF AP across n partitions (step-0 partition dim)."""
    return bass.AP(tensor=ap.tensor, offset=ap.offset, ap=[[0, n]] + list(ap.ap)[1:])


def _bcast_free(ap_col, n):
    """Broadcast a (128, 1) AP along the free axis with step 0."""
    a = list(ap_col.ap)
    return bass.AP(tensor=ap_col.tensor, offset=ap_col.offset,
                   ap=[a[0], [0, n]])


def _bcast_mid(ap, reps, pos=1):
    """Insert a step-0 dim of size `reps` at position `pos` of an AP."""
    a = list(ap.ap)
    return bass.AP(tensor=ap.tensor, offset=ap.offset,
                   ap=a[:pos] + [[0, reps]] + a[pos:])




_CACHE = {}


def _get_nc(t_len=T):
    if t_len not in _CACHE:
        _CACHE[t_len] = build_nc(t_len)
    return _CACHE[t_len]


def make_in_maps(x, rotary_pos_emb, ln1_w, w_qkv, qn_w, kn_w, w_out, ln2_w,
                 w_fc1, w_fc2, t_len=T):
    """Host-side sharding prep. Returns list of per-core input dicts."""
    TL = t_len
    x = np.asarray(x, np.float32)
    rot = np.asarray(rotary_pos_emb, np.float32)
    cosd = np.cos(rot).astype(np.float32)
    sin = np.sin(rot).astype(np.float32)
    sinneg = np.concatenate([-sin[:, :64], sin[:, :64]], axis=-1).astype(np.float32)
    w_qkv_f = (np.asarray(w_qkv, np.float32)
               * np.asarray(ln1_w, np.float32)[:, None]).reshape(C, 3, H, D)
    w_fc1_f = (np.asarray(w_fc1, np.float32)
               * np.asarray(ln2_w, np.float32)[:, None])
    w_fc2_b = np.asarray(w_fc2, np.float32).astype(ml_dtypes.bfloat16)
    wo = np.asarray(w_out, np.float32).reshape(H, D, C)
    qn = np.asarray(qn_w, np.float32).reshape(1, D)
    kn = np.asarray(kn_w, np.float32).reshape(1, D)

    in_maps = []
    for c in range(N_CORES):
        b, hg = c // 2, c % 2
        heads = slice(hg * HPC, (hg + 1) * HPC)
        pheads = slice((1 - hg) * HPC, (2 - hg) * HPC)
        wq = np.ascontiguousarray(
            w_qkv_f[:, :, heads, :].reshape(C, 3 * CPC)).astype(ml_dtypes.bfloat16)
        w_outp = np.concatenate([wo[heads].reshape(CPC, C),
                                 wo[pheads].reshape(CPC, C)],
                                axis=0).astype(ml_dtypes.bfloat16)
        sel = np.zeros(16, np.float32)
        sel[0] = 1.0 - hg   # l0
        sel[1] = float(hg)  # l1
        sel[2] = float(hg)      # s0: send peer half
        sel[3] = 1.0 - hg       # s1
        peer = c ^ 1
        sel[4 + peer] = 1.0
        in_maps.append({
            'x': np.ascontiguousarray(x[b, :TL]),
            'xh': np.ascontiguousarray(x[b, hg * TL // 2:(hg + 1) * TL // 2]),
            'w_qkv': wq,
            'cosd': np.ascontiguousarray(cosd[:TL]),
            'sinneg': np.ascontiguousarray(sinneg[:TL]),
            'qn': qn, 'kn': kn,
            'selv': sel.reshape(1, 16),
            'w_out': np.ascontiguousarray(w_outp),
            'w_fc1': np.ascontiguousarray(w_fc1_f).astype(ml_dtypes.bfloat16),
            'w_fc2': np.ascontiguousarray(w_fc2_b),
        })
    return in_maps


def assemble_output(results, t_len=T):
    out = np.zeros((B, t_len, C), np.float32)
    for c in range(N_CORES):
        b, hg = c // 2, c % 2
        out[b, hg * t_len // 2:(hg + 1) * t_len // 2] = results[c]['y']
    return out


def kernel(**inputs):
    nc = _get_nc(T)
    in_maps = make_in_maps(**inputs)
    res = bass_utils.run_bass_kernel_spmd(nc, in_maps,
                                          core_ids=list(range(N_CORES)))
    return assemble_output(res.results)
def build_nc(t_len=T, n_cores=N_CORES, no_collective=False):
    """Build + compile the SPMD kernel graph for per-core sequence length t_len."""
    import contextlib
    TL = t_len
    NT = TL // 128          # token tiles (full T)
    NH = TL // 2 // 128     # token tiles of my half
    QB = TL // 512          # 512-wide query blocks
    NB = D // 32            # 32-blocks per head (4)
    NBLK = HPC * NB         # qdq blocks per tensor (20)
    inv_sqrt_d = float(1.0 / np.sqrt(D))

    nc = bacc.Bacc('TRN2', target_bir_lowering=False, debug=False,
                   num_devices=n_cores)

    # ---- DRAM I/O ----
    x_d = nc.dram_tensor('x', [TL, C], f32, kind='ExternalInput')
    xh_d = nc.dram_tensor('xh', [TL // 2, C], f32, kind='ExternalInput')
    wqkv_d = nc.dram_tensor('w_qkv', [C, 3 * CPC], bf16, kind='ExternalInput')
    cos_d = nc.dram_tensor('cosd', [TL, D], f32, kind='ExternalInput')
    sinn_d = nc.dram_tensor('sinneg', [TL, D], f32, kind='ExternalInput')
    qn_d = nc.dram_tensor('qn', [1, D], f32, kind='ExternalInput')
    kn_d = nc.dram_tensor('kn', [1, D], f32, kind='ExternalInput')
    sel_d = nc.dram_tensor('selv', [1, 16], f32, kind='ExternalInput')
    wout_d = nc.dram_tensor('w_out', [C, C], bf16, kind='ExternalInput')
    wfc1_d = nc.dram_tensor('w_fc1', [C, F], bf16, kind='ExternalInput')
    wfc2_d = nc.dram_tensor('w_fc2', [F, C], bf16, kind='ExternalInput')
    y_d = nc.dram_tensor('y', [TL // 2, C], f32, kind='ExternalOutput')

    with tile.TileContext(nc) as tc:
        with contextlib.ExitStack() as ctx:
            persist = ctx.enter_context(tc.tile_pool(name='persist', bufs=1))
            dram = ctx.enter_context(tc.tile_pool(name='dram', bufs=1, space='DRAM'))

            # ---- constants (persist) ----
            ident_b = persist.tile([128, 128], bf16)
            make_identity(nc, ident_b)
            ident_f = persist.tile([128, 128], f32)
            make_identity(nc, ident_f)
            sel_sb = persist.tile([128, 16], f32)
            nc.sync.dma_start(out=sel_sb[:], in_=_bcast(sel_d.ap()))
            zero_sb = persist.tile([128, 1], f32)
            nc.vector.memset(zero_sb[:], 0.0)
            eps_sb = persist.tile([128, 1], f32)
            nc.vector.memset(eps_sb[:], EPS)

            # ---- DRAM scratch for phase-boundary tensors ----
            attn_dram = dram.tile([TL, CPC], f32)
            loc_dram = dram.tile([TL // 2, CPC], bf16)
            peer_dram = dram.tile([TL // 2, CPC], bf16)
            x2_dram = dram.tile([TL // 2, C], f32)
            bounce_in = dram.tile([TL // 2, CPC], bf16)
            ag_outA = dram.tile([n_cores * TL // 4, CPC], bf16, addr_space='Shared')
            ag_outB = dram.tile([n_cores * TL // 4, CPC], bf16, addr_space='Shared')

            # =========== Phases A+B: qT/kT/vd live across both ===============
            with tc.tile_pool(name='ab', bufs=1) as ab:
                qT = ab.tile([128, HPC, TL], bf16)
                kT = ab.tile([128, HPC, TL], bf16)
                vd_sb = ab.tile([128, NT, HPC, D + 1], bf16)

                # ---------------- Phase A ------------------------------------
                with contextlib.ExitStack() as pa:
                    a_w = pa.enter_context(tc.tile_pool(name='a_w', bufs=1))
                    a_tmp = pa.enter_context(tc.tile_pool(name='a_tmp', bufs=3))
                    a_tm2 = pa.enter_context(tc.tile_pool(name='a_tm2', bufs=2))
                    a_qdq = pa.enter_context(tc.tile_pool(name='a_qdq', bufs=4))
                    a_ps = pa.enter_context(
                        tc.tile_pool(name='a_ps', bufs=3, space='PSUM'))
                    tr_ps = pa.enter_context(
                        tc.tile_pool(name='tr_ps', bufs=2, space='PSUM'))

                    qn_sb = a_w.tile([128, D], f32)
                    nc.sync.dma_start(out=qn_sb[:], in_=_bcast(qn_d.ap()))
                    kn_sb = a_w.tile([128, D], f32)
                    nc.sync.dma_start(out=kn_sb[:], in_=_bcast(kn_d.ap()))
                    wq_sb = a_w.tile([128, 10, 3 * CPC], bf16)
                    nc.sync.dma_start(
                        out=wq_sb[:],
                        in_=wqkv_d.ap().rearrange('(j p) c -> p j c', p=128))

                    def blk_bcast(ap2):
                        # (128, NBLK) -> (128, HPC, NB, 32), 0-step last dim
                        a = list(ap2.ap)
                        st = a[1][0]
                        return bass.AP(tensor=ap2.tensor, offset=ap2.offset,
                                       ap=[a[0], [st * NB, HPC], [st, NB], [0, 32]])

                    def v4(ap3):
                        return ap3.rearrange('p h (b e) -> p h b e', e=32)

                    def qdq(eng_a, eng_b, src4, dst4, blk_tag):
                        # src4/dst4: (128, HPC, NB, 32) APs (src psum or sbuf)
                        amax = a_qdq.tile([128, NBLK], f32, tag=blk_tag + 'am')
                        eng_a.tensor_reduce(out=amax[:], in_=src4, axis=AX.X,
                                            op=OP.max, apply_absolute_value=True)
                        eng_a.tensor_scalar_max(out=amax[:], in0=amax[:],
                                                scalar1=1e-12)
                        eb = a_qdq.tile([128, NBLK], i32, tag=blk_tag + 'eb')
                        eng_a.tensor_single_scalar(out=eb[:],
                                                   in_=amax[:].bitcast(i32),
                                                   scalar=23,
                                                   op=OP.logical_shift_right)
                        f2 = a_qdq.tile([128, NBLK], i32, tag=blk_tag + 'f2')
                        eng_a.tensor_scalar(out=f2[:], in0=eb[:], scalar1=-1,
                                            scalar2=260, op0=OP.mult, op1=OP.add)
                        sc = a_qdq.tile([128, NBLK], f32, tag=blk_tag + 'sc')
                        eng_a.tensor_single_scalar(out=sc[:].bitcast(i32),
                                                   in_=f2[:], scalar=23,
                                                   op=OP.logical_shift_left)
                        eng_a.tensor_single_scalar(out=eb[:], in_=eb[:], scalar=6,
                                                   op=OP.subtract)
                        isc = a_qdq.tile([128, NBLK], f32, tag=blk_tag + 'is')
                        eng_a.tensor_single_scalar(out=isc[:].bitcast(i32),
                                                   in_=eb[:], scalar=23,
                                                   op=OP.logical_shift_left)
                        ys = a_qdq.tile([128, HPC, NB, 32], f32, tag='ys')
                        eng_b.tensor_tensor(out=ys[:], in0=src4,
                                            in1=blk_bcast(sc[:]), op=OP.mult)
                        q8 = a_qdq.tile([128, HPC, NB, 32], fp8, tag='q8')
                        eng_b.tensor_scalar(out=q8[:], in0=ys[:], scalar1=-112.0,
                                            scalar2=112.0, op0=OP.max, op1=OP.min)
                        eng_b.tensor_tensor(out=dst4, in0=q8[:],
                                            in1=blk_bcast(isc[:]), op=OP.mult)

                    pending_T = []

                    def flush_T(upto):
                        while pending_T and pending_T[0][0] <= upto:
                            _, qd_p, kd_p, t_p = pending_T.pop(0)
                            for h in range(HPC):
                                for src_, dstT in ((qd_p, qT), (kd_p, kT)):
                                    tp2 = tr_ps.tile([128, 128], bf16, tag='tp')
                                    nc.tensor.transpose(tp2[:], src_[:, h, :],
                                                        ident_b[:])
                                    nc.any.tensor_copy(
                                        out=dstT[:, h, t_p * 128:(t_p + 1) * 128],
                                        in_=tp2[:])

                    stash = {}

                    def emit_head(t):
                        xt = a_tm2.tile([128, C], f32, tag='xt')
                        nc.sync.dma_start(out=xt[:],
                                          in_=x_d[t * 128:(t + 1) * 128, :])
                        cos_t = a_tmp.tile([128, D], f32, tag='cos_t')
                        nc.sync.dma_start(out=cos_t[:],
                                          in_=cos_d[t * 128:(t + 1) * 128, :])
                        sinn_t = a_tmp.tile([128, D], f32, tag='sinn_t')
                        nc.sync.dma_start(out=sinn_t[:],
                                          in_=sinn_d[t * 128:(t + 1) * 128, :])
                        trash = a_tm2.tile([128, C], bf16, tag='trash')
                        ssq = a_tm2.tile([128, 1], f32, tag='ssq')
                        nc.scalar.activation(out=trash[:], in_=xt[:],
                                             func=AF.Square, bias=zero_sb[:],
                                             accum_out=ssq[:])
                        rstd = a_tm2.tile([128, 1], f32, tag='rstd')
                        nc.scalar.activation(out=rstd[:], in_=ssq[:], func=AF.Sqrt,
                                             scale=float(1.0 / C), bias=eps_sb[:])
                        nc.vector.reciprocal(out=rstd[:], in_=rstd[:])
                        xn = a_tm2.tile([128, C], bf16, tag='xn')
                        nc.scalar.activation(out=xn[:], in_=xt[:], func=AF.Copy,
                                             scale=rstd[:])
                        xnT = a_tm2.tile([128, 10, 128], bf16, tag='xnT')
                        for j in range(10):
                            tp = tr_ps.tile([128, 128], bf16, tag='tp')
                            nc.tensor.transpose(tp[:], xn[:, j * 128:(j + 1) * 128],
                                                ident_b[:])
                            nc.any.tensor_copy(out=xnT[:, j, :], in_=tp[:])
                        sbufs = []
                        for g in range(3):
                            ps = a_ps.tile([128, CPC], f32, tag='qkv_ps')
                            for lo, hi in ((0, 512), (512, CPC)):
                                for j in range(10):
                                    nc.tensor.matmul(
                                        ps[:, lo:hi], xnT[:, j, :],
                                        wq_sb[:, j, g * CPC + lo:g * CPC + hi],
                                        start=(j == 0), stop=(j == 9))
                            sb = a_tmp.tile([128, CPC], f32,
                                            tag='qkv_sb' + str(g))
                            nc.scalar.copy(out=sb[:], in_=ps[:])
                            sbufs.append(sb)
                        stash[t] = (sbufs, cos_t, sinn_t)

                    def emit_tail(t):
                        (q_sb, k_sb, v_sb), cos_t, sinn_t = stash.pop(t)

                        def rope(eng, src, out, tmp):
                            src3 = src.rearrange('p (h d) -> p h d', h=HPC)
                            swap = bass.AP(tensor=src3.tensor,
                                           offset=src3.offset + 64,
                                           ap=list(src3.ap)[:2] + [[-64, 2],
                                                                   [1, 64]])
                            sin4 = bass.AP(tensor=sinn_t.tensor,
                                           offset=sinn_t[:].offset,
                                           ap=[list(sinn_t[:].ap)[0], [0, HPC],
                                               [64, 2], [1, 64]])
                            eng.tensor_tensor(
                                out=tmp[:].rearrange('p h (u d) -> p h u d', u=2),
                                in0=swap, in1=sin4, op=OP.mult)
                            eng.tensor_tensor(out=out[:], in0=src3,
                                              in1=_bcast_mid(cos_t[:], HPC),
                                              op=OP.mult)
                            eng.tensor_add(out=out[:], in0=out[:], in1=tmp[:])

                        def qknorm(app_eng, r, w_sb, sq_tag):
                            sqs = a_tm2.tile([128, HPC, D], f32, tag='scr')
                            nc.scalar.activation(out=sqs[:], in_=r[:],
                                                 func=AF.Square, bias=zero_sb[:])
                            ms = a_tm2.tile([128, HPC], f32, tag=sq_tag + 'ms')
                            nc.vector.tensor_reduce(out=ms[:], in_=sqs[:],
                                                    axis=AX.X, op=OP.add)
                            nc.scalar.activation(out=ms[:], in_=ms[:], func=AF.Sqrt,
                                                 scale=float(1.0 / D),
                                                 bias=eps_sb[:])
                            nc.vector.reciprocal(out=ms[:], in_=ms[:])
                            if app_eng is nc.vector:
                                for h in range(HPC):
                                    app_eng.scalar_tensor_tensor(
                                        out=r[:, h, :], in0=r[:, h, :],
                                        scalar=ms[:, h:h + 1], in1=w_sb[:],
                                        op0=OP.mult, op1=OP.mult)
                            else:
                                for h in range(HPC):
                                    app_eng.tensor_tensor(
                                        out=r[:, h, :], in0=r[:, h, :],
                                        in1=_bcast_free(ms[:, h:h + 1], D),
                                        op=OP.mult)
                                app_eng.tensor_tensor(
                                    out=r[:], in0=r[:],
                                    in1=_bcast_mid(w_sb[:], HPC), op=OP.mult)

                        qr = a_tm2.tile([128, HPC, D], f32, tag='qr')
                        rtmp = a_tm2.tile([128, HPC, D], f32, tag='rtmp')
                        rope(nc.vector, q_sb[:], qr, rtmp)
                        qknorm(nc.vector, qr, qn_sb, 'q')
                        qd = a_qdq.tile([128, HPC, D], bf16, tag='qd')
                        qdq(nc.vector, nc.vector, v4(qr[:]), v4(qd[:]), 'q')
                        kr = a_tm2.tile([128, HPC, D], f32, tag='kr')
                        ktmp = a_tm2.tile([128, HPC, D], f32, tag='ktmp')
                        rope(nc.gpsimd, k_sb[:], kr, ktmp)
                        qknorm(nc.gpsimd, kr, kn_sb, 'k')
                        kd = a_qdq.tile([128, HPC, D], bf16, tag='kd')
                        qdq(nc.vector, nc.gpsimd, v4(kr[:]), v4(kd[:]), 'k')
                        qdq(nc.vector, nc.vector,
                            v4(v_sb[:].rearrange('p (h d) -> p h d', h=HPC)),
                            v4(vd_sb[:, t, :, 0:D]), 'v')
                        nc.vector.memset(vd_sb[:, t, :, D:D + 1], 1.0)
                        pending_T.append((t, qd, kd, t))

                    for t in range(NT):
                        emit_head(t)
                        if t >= 1:
                            emit_tail(t - 1)
                        flush_T(t - 3)
                    emit_tail(NT - 1)
                    flush_T(NT)
                # ---------------- Phase B: attention -------------------------
                with contextlib.ExitStack() as pb:
                    b_tmp = pb.enter_context(tc.tile_pool(name='b_tmp', bufs=3))
                    pT_pool = pb.enter_context(
                        tc.tile_pool(name='pT', bufs=4 * QB + 3))
                    s_ps = pb.enter_context(
                        tc.tile_pool(name='s_ps', bufs=2, space='PSUM'))
                    o_ps = pb.enter_context(
                        tc.tile_pool(name='o_ps', bufs=3, space='PSUM'))

                    for qb in range(QB):
                        for h in range(HPC):
                            nkt = 4 * qb + 4
                            pTs = []
                            for kt in range(nkt):
                                sp = s_ps.tile([128, 512], f32, tag='sp')
                                nc.tensor.matmul(
                                    sp[:], kT[:, h, kt * 128:(kt + 1) * 128],
                                    qT[:, h, qb * 512:(qb + 1) * 512],
                                    start=True, stop=True)
                                pT = pT_pool.tile([128, 512], bf16, tag='pT')
                                nc.scalar.activation(out=pT[:], in_=sp[:],
                                                     func=AF.Exp, bias=zero_sb[:],
                                                     scale=inv_sqrt_d)
                                o = kt - 4 * qb
                                if o >= 0:
                                    nc.gpsimd.affine_select(
                                        out=pT[:], in_=pT[:], compare_op=OP.is_ge,
                                        fill=0.0, base=-128 * o,
                                        pattern=[[1, 512]], channel_multiplier=-1)
                                pTs.append(pT)
                            for ql in range(4):
                                qt = qb * 4 + ql
                                op = o_ps.tile([128, D + 1], f32, tag='op')
                                for kt in range(qt + 1):
                                    nc.tensor.matmul(
                                        op[:],
                                        pTs[kt][:, ql * 128:(ql + 1) * 128],
                                        vd_sb[:, kt, h, :],
                                        start=(kt == 0), stop=(kt == qt))
                                rc = b_tmp.tile([128, 1], f32, tag='rc')
                                nc.vector.reciprocal(out=rc[:], in_=op[:, D:D + 1])
                                anorm = b_tmp.tile([128, D], f32, tag='anorm')
                                nc.vector.tensor_scalar_mul(
                                    out=anorm[:], in0=op[:, 0:D], scalar1=rc[:])
                                nc.sync.dma_start(
                                    out=attn_dram[qt * 128:(qt + 1) * 128,
                                                  h * D:(h + 1) * D],
                                    in_=anorm[:])

            # ============ Phase C: exchange + out_proj =======================
            # C1: masked local/send halves -> DRAM / AG bounce (two halves,
            # each AllGather overlaps the remaining attention work)
            NQ = NH // 2
            with tc.tile_pool(name='c1', bufs=3) as c1:
                def emit_c1(j):
                    aj = c1.tile([128, CPC], f32, tag='aj')
                    nc.sync.dma_start(out=aj[:],
                                      in_=attn_dram[j * 128:(j + 1) * 128, :])
                    ajn = c1.tile([128, CPC], f32, tag='ajn')
                    nc.sync.dma_start(
                        out=ajn[:],
                        in_=attn_dram[(j + NH) * 128:(j + NH + 1) * 128, :])
                    locj = c1.tile([128, CPC], bf16, tag='locj')
                    nc.vector.tensor_scalar_mul(out=locj[:], in0=aj[:],
                                                scalar1=sel_sb[:, 0:1])
                    nc.vector.scalar_tensor_tensor(out=locj[:], in0=ajn[:],
                                                   scalar=sel_sb[:, 1:2],
                                                   in1=locj[:],
                                                   op0=OP.mult, op1=OP.add)
                    nc.sync.dma_start(out=loc_dram[j * 128:(j + 1) * 128, :],
                                      in_=locj[:])
                    sndj = c1.tile([128, CPC], bf16, tag='sndj')
                    sndt = c1.tile([128, CPC], bf16, tag='sndt')
                    nc.gpsimd.tensor_tensor(out=sndj[:], in0=aj[:],
                                            in1=_bcast_free(sel_sb[:, 2:3], CPC),
                                            op=OP.mult)
                    nc.gpsimd.tensor_tensor(out=sndt[:], in0=ajn[:],
                                            in1=_bcast_free(sel_sb[:, 3:4], CPC),
                                            op=OP.mult)
                    nc.gpsimd.tensor_add(out=sndj[:], in0=sndj[:], in1=sndt[:])
                    nc.sync.dma_start(out=bounce_in[j * 128:(j + 1) * 128, :],
                                      in_=sndj[:])

                for j in range(NQ):
                    emit_c1(j)
                nc.gpsimd.collective_compute(
                    'AllGather', OP.bypass,
                    ins=[bounce_in[0:TL // 4, :].opt()],
                    outs=[ag_outA[:].opt()],
                    replica_groups=[list(range(n_cores))])
                for j in range(NQ, NH):
                    emit_c1(j)
                nc.gpsimd.collective_compute(
                    'AllGather', OP.bypass,
                    ins=[bounce_in[TL // 4:TL // 2, :].opt()],
                    outs=[ag_outB[:].opt()],
                    replica_groups=[list(range(n_cores))])
            # one-hot extraction of peer blocks -> peer_dram
            wo_pool_cm = tc.tile_pool(name='wo', bufs=1)
            wo_pool = wo_pool_cm.__enter__()
            wo_sb = wo_pool.tile([128, 10, C], bf16)
            nc.sync.dma_start(
                out=wo_sb[:],
                in_=wout_d.ap().rearrange('(j p) c -> p j c', p=128))
            with tc.tile_pool(name='cpe', bufs=3) as cpe, \
                 tc.tile_pool(name='cpa', bufs=1) as cpa:
                for half, ago in ((0, ag_outA), (1, ag_outB)):
                    peer = cpa.tile([128, NQ, CPC], bf16, tag='peer')
                    for r in range(n_cores):
                        blk = cpe.tile([128, NQ, CPC], bf16, tag='agblk')
                        nc.gpsimd.dma_start(
                            out=blk[:],
                            in_=ago[r * TL // 4:(r + 1) * TL // 4, :]
                            .rearrange('(j p) c -> p j c', p=128))
                        if r == 0:
                            nc.vector.tensor_scalar_mul(out=peer[:], in0=blk[:],
                                                        scalar1=sel_sb[:, 4:5])
                        else:
                            nc.vector.scalar_tensor_tensor(
                                out=peer[:], in0=blk[:],
                                scalar=sel_sb[:, 4 + r:5 + r], in1=peer[:],
                                op0=OP.mult, op1=OP.add)
                    nc.sync.dma_start(
                        out=peer_dram[half * TL // 4:(half + 1) * TL // 4, :]
                        .rearrange('(j p) c -> p j c', p=128),
                        in_=peer[:])
            # C2: out_proj (w_out fully resident, tt-outer)
            with tc.tile_pool(name='c2', bufs=2) as c2, \
                 tc.tile_pool(name='c_ps', bufs=3, space='PSUM') as c_ps, \
                 tc.tile_pool(name='ct_ps', bufs=2, space='PSUM') as ct_ps:
                for tt in range(NH):
                    lct = c2.tile([128, CPC], bf16, tag='lct')
                    nc.sync.dma_start(out=lct[:],
                                      in_=loc_dram[tt * 128:(tt + 1) * 128, :])
                    pct = c2.tile([128, CPC], bf16, tag='pct')
                    nc.sync.dma_start(out=pct[:],
                                      in_=peer_dram[tt * 128:(tt + 1) * 128, :])
                    lT = c2.tile([128, 10, 128], bf16, tag='lT')
                    for ci in range(HPC):
                        tpl = ct_ps.tile([128, 128], bf16, tag='tpl')
                        nc.tensor.transpose(tpl[:],
                                            lct[:, ci * 128:(ci + 1) * 128],
                                            ident_b[:])
                        nc.any.tensor_copy(out=lT[:, ci, :], in_=tpl[:])
                        tpp = ct_ps.tile([128, 128], bf16, tag='tpp')
                        nc.tensor.transpose(tpp[:],
                                            pct[:, ci * 128:(ci + 1) * 128],
                                            ident_b[:])
                        nc.any.tensor_copy(out=lT[:, HPC + ci, :], in_=tpp[:])
                    for lo, hi in ((0, 512), (512, 1024), (1024, C)):
                        ps = c_ps.tile([128, 512], f32, tag='oproj_ps')
                        for jj in range(10):
                            nc.tensor.matmul(
                                ps[:, 0:hi - lo],
                                lT[:, jj, :],
                                wo_sb[:, jj, lo:hi],
                                start=(jj == 0), stop=(jj == 9))
                        xht = c2.tile([128, 512], f32, tag='xht')
                        nc.sync.dma_start(
                            out=xht[:, 0:hi - lo],
                            in_=xh_d[tt * 128:(tt + 1) * 128, lo:hi])
                        x2t = c2.tile([128, 512], f32, tag='x2t')
                        nc.vector.tensor_add(out=x2t[:, 0:hi - lo],
                                             in0=ps[:, 0:hi - lo],
                                             in1=xht[:, 0:hi - lo])
                        nc.sync.dma_start(
                            out=x2_dram[tt * 128:(tt + 1) * 128, lo:hi],
                            in_=x2t[:, 0:hi - lo])
            wo_pool_cm.__exit__(None, None, None)
            # ================= Phase D: MLP ==================================
            with contextlib.ExitStack() as pd:
                d_tmp = pd.enter_context(tc.tile_pool(name='d_tmp', bufs=3))
                h2_pool = pd.enter_context(tc.tile_pool(name='h2', bufs=1))
                h2T = h2_pool.tile([128, F // 128, TL // 2], bf16)
                tchunks = [(s, min(s + 512, TL // 2))
                           for s in range(0, TL // 2, 512)]

                with tc.tile_pool(name='xn2', bufs=1) as xn2_pool, \
                     tc.tile_pool(name='dt_ps', bufs=2, space='PSUM') as dt_ps, \
                     tc.tile_pool(name='h_ps', bufs=3, space='PSUM') as h_ps, \
                     tc.tile_pool(name='wf1', bufs=3) as wf1_pool:
                    xn2T = xn2_pool.tile([128, 10, TL // 2], bf16)
                    for tt in range(NH):
                        x2t = d_tmp.tile([128, C], f32, tag='x2ld')
                        nc.sync.dma_start(out=x2t[:],
                                          in_=x2_dram[tt * 128:(tt + 1) * 128, :])
                        trash2 = d_tmp.tile([128, C], bf16, tag='trash2')
                        ssq = d_tmp.tile([128, 1], f32, tag='ssq2')
                        nc.scalar.activation(out=trash2[:], in_=x2t[:],
                                             func=AF.Square, bias=zero_sb[:],
                                             accum_out=ssq[:])
                        rstd = d_tmp.tile([128, 1], f32, tag='rstd2')
                        nc.scalar.activation(out=rstd[:], in_=ssq[:], func=AF.Sqrt,
                                             scale=float(1.0 / C), bias=eps_sb[:])
                        nc.vector.reciprocal(out=rstd[:], in_=rstd[:])
                        xn2 = d_tmp.tile([128, C], bf16, tag='xn2t')
                        nc.scalar.activation(out=xn2[:], in_=x2t[:], func=AF.Copy,
                                             scale=rstd[:])
                        for j in range(10):
                            tp = dt_ps.tile([128, 128], bf16, tag='xn2_tp')
                            nc.tensor.transpose(tp[:],
                                                xn2[:, j * 128:(j + 1) * 128],
                                                ident_b[:])
                            nc.any.tensor_copy(
                                out=xn2T[:, j, tt * 128:(tt + 1) * 128],
                                in_=tp[:])

                    for fi in range(F // 128):
                        wf1 = wf1_pool.tile([128, 10, 128], bf16, tag='wf1')
                        nc.sync.dma_start(
                            out=wf1[:],
                            in_=wfc1_d[:, fi * 128:(fi + 1) * 128]
                            .rearrange('(j p) c -> p j c', p=128))
                        for clo, chi in tchunks:
                            hps = h_ps.tile([128, 512], f32, tag='hps')
                            for j in range(10):
                                nc.tensor.matmul(
                                    hps[:, 0:chi - clo],
                                    wf1[:, j, :],
                                    xn2T[:, j, clo:chi],
                                    start=(j == 0), stop=(j == 9))
                            hrelu = d_tmp.tile([128, 512], bf16, tag='hrelu')
                            nc.scalar.activation(out=hrelu[:, 0:chi - clo],
                                                 in_=hps[:, 0:chi - clo],
                                                 func=AF.Relu, bias=zero_sb[:])
                            nc.vector.tensor_mul(out=h2T[:, fi, clo:chi],
                                                 in0=hrelu[:, 0:chi - clo],
                                                 in1=hrelu[:, 0:chi - clo])

                with tc.tile_pool(name='y_ps', bufs=NH, space='PSUM') as y_ps, \
                     tc.tile_pool(name='wf2', bufs=3) as wf2_pool:
                    for lo, hi in ((0, 512), (512, 1024), (1024, C)):
                        yps = []
                        for _i in range(NH):
                            ypt = y_ps.tile([128, 512], f32, tag='yps')
                            yps.append(ypt)
                        for fi in range(F // 128):
                            wf2 = wf2_pool.tile([128, 512], bf16, tag='wf2')
                            nc.sync.dma_start(
                                out=wf2[:, 0:hi - lo],
                                in_=wfc2_d[fi * 128:(fi + 1) * 128, lo:hi])
                            for tt in range(NH):
                                nc.tensor.matmul(
                                    yps[tt][:, 0:hi - lo],
                                    h2T[:, fi, tt * 128:(tt + 1) * 128],
                                    wf2[:, 0:hi - lo],
                                    start=(fi == 0), stop=(fi == F // 128 - 1))
                        for tt in range(NH):
                            x2s = d_tmp.tile([128, 512], f32, tag='x2s')
                            nc.sync.dma_start(
                                out=x2s[:, 0:hi - lo],
                                in_=x2_dram[tt * 128:(tt + 1) * 128, lo:hi])
                            yo = d_tmp.tile([128, 512], f32, tag='yo')
                            nc.vector.tensor_add(out=yo[:, 0:hi - lo],
                                                 in0=yps[tt][:, 0:hi - lo],
                                                 in1=x2s[:, 0:hi - lo])
                            nc.sync.dma_start(
                                out=y_d[tt * 128:(tt + 1) * 128, lo:hi],
                                in_=yo[:, 0:hi - lo])

    nc.compile()
    return nc


_CACHE = {}


def _get_nc(t_len=T):
    if t_len not in _CACHE:
        _CACHE[t_len] = build_nc(t_len)
    return _CACHE[t_len]


def make_in_maps(x, rotary_pos_emb, ln1_w, w_qkv, qn_w, kn_w, w_out, ln2_w,
                 w_fc1, w_fc2, t_len=T):
    """Host-side sharding prep. Returns list of per-core input dicts."""
    TL = t_len
    x = np.asarray(x, np.float32)
    rot = np.asarray(rotary_pos_emb, np.float32)
    cosd = np.cos(rot).astype(np.float32)
    sin = np.sin(rot).astype(np.float32)
    sinneg = np.concatenate([-sin[:, :64], sin[:, :64]], axis=-1).astype(np.float32)
    w_qkv_f = (np.asarray(w_qkv, np.float32)
               * np.asarray(ln1_w, np.float32)[:, None]).reshape(C, 3, H, D)
    w_fc1_f = (np.asarray(w_fc1, np.float32)
               * np.asarray(ln2_w, np.float32)[:, None])
    w_fc2_b = np.asarray(w_fc2, np.float32).astype(ml_dtypes.bfloat16)
    wo = np.asarray(w_out, np.float32).reshape(H, D, C)
    qn = np.asarray(qn_w, np.float32).reshape(1, D)
    kn = np.asarray(kn_w, np.float32).reshape(1, D)

    in_maps = []
    for c in range(N_CORES):
        b, hg = c // 2, c % 2
        heads = slice(hg * HPC, (hg + 1) * HPC)
        pheads = slice((1 - hg) * HPC, (2 - hg) * HPC)
        wq = np.ascontiguousarray(
            w_qkv_f[:, :, heads, :].reshape(C, 3 * CPC)).astype(ml_dtypes.bfloat16)
        w_outp = np.concatenate([wo[heads].reshape(CPC, C),
                                 wo[pheads].reshape(CPC, C)],
                                axis=0).astype(ml_dtypes.bfloat16)
        sel = np.zeros(16, np.float32)
        sel[0] = 1.0 - hg   # l0
        sel[1] = float(hg)  # l1
        sel[2] = float(hg)      # s0: send peer half
        sel[3] = 1.0 - hg       # s1
        peer = c ^ 1
        sel[4 + peer] = 1.0
        in_maps.append({
            'x': np.ascontiguousarray(x[b, :TL]),
            'xh': np.ascontiguousarray(x[b, hg * TL // 2:(hg + 1) * TL // 2]),
            'w_qkv': wq,
            'cosd': np.ascontiguousarray(cosd[:TL]),
            'sinneg': np.ascontiguousarray(sinneg[:TL]),
            'qn': qn, 'kn': kn,
            'selv': sel.reshape(1, 16),
            'w_out': np.ascontiguousarray(w_outp),
            'w_fc1': np.ascontiguousarray(w_fc1_f).astype(ml_dtypes.bfloat16),
            'w_fc2': np.ascontiguousarray(w_fc2_b),
        })
    return in_maps


def assemble_output(results, t_len=T):
    out = np.zeros((B, t_len, C), np.float32)
    for c in range(N_CORES):
        b, hg = c // 2, c % 2
        out[b, hg * t_len // 2:(hg + 1) * t_len // 2] = results[c]['y']
    return out


def kernel(**inputs):
    nc = _get_nc(T)
    in_maps = make_in_maps(**inputs)
    res = bass_utils.run_bass_kernel_spmd(nc, in_maps,
                                          core_ids=list(range(N_CORES)))
    return assemble_output(res.results)

